# revision 20
# baseline (speedup 1.0000x reference)
"""Multi-head attention (B=4, T=2048, C=1024, H=16) on 8 trn2 NeuronCores.

Sharding: core c -> (batch b = c//2, head-half = c%2, 8 heads each).
Each core computes its 8 heads' QKV projections, full attention over
T=2048, and a *partial* output projection (contraction over its 512
merged channels).  The host sums the two partials per batch and adds
the output bias (the "all-reduce after the output projection" done at
unshard time, host-side).

Device layout notes:
  - all matmuls run as float32r (full-rate fp32 on the PE array);
    every tensor feeding a matmul is declared float32r so producers
    round on write (walrus requires it)
  - scores are computed transposed St[tk, tq] so softmax needs no
    partition-axis reduction: exp runs elementwise on ACT, the
    denominator comes from a ones-column appended to the V tiles
    (PV matmul yields numerator rows 0..63 and denominator row 64)
  - no max-subtraction in softmax: scores ~ N(0,1), exp stays in fp32
    range and matches the reference numerically
  - k/v projections run up front; the q projection, attention, and
    output projection are interleaved per 512-wide query chunk so the
    ACT engine (the softmax-exp bottleneck) starts early and PE work
    fills its shadow
"""

import math
import numpy as np
from contextlib import ExitStack

import concourse.bass as bass
import concourse.tile as tile
from concourse import bacc, mybir
from concourse import bass_utils

P = 128
F32 = mybir.dt.float32
F32R = mybir.dt.float32r

D_MODEL = 1024
N_HEAD = 16
HEAD_DIM = 64
B = 4
T_FULL = 2048
CH = D_MODEL // 2          # per-core merged-channel block (8 heads * 64)
N_CORES = 8


def emit_mha(tc, outT, qT, kT, vT, wqT, wkT, wvT, woT, *,
             C, T, CHL, HD, TQ=512):
    nc = tc.nc
    NC_T = C // P            # contraction tiles for qkv projections
    NO_T = CHL // P          # o-tiles of the local head block
    NTK = T // P             # key tiles
    NQ = T // TQ             # query chunks
    H = CHL // HD            # local heads
    HPT = P // HD            # heads per 128-row tile (2)
    NFO = C // P             # full-C o-tiles for the output projection
    TKC = TQ // P            # tk tiles per input chunk
    ExpF = mybir.ActivationFunctionType.Exp
    scale = 1.0 / math.sqrt(HD)

    dma_engines = [nc.sync, nc.gpsimd, nc.scalar]

    def load_split(dst, src_ap):
        """DMA a (C_sub*P, F) DRAM block into dst [P, C_sub, F], one DMA
        per c-tile spread over queues so compute can start after the
        first slice lands and transfers run in parallel."""
        r = src_ap.rearrange("(c p) f -> p c f", p=P)
        for c in range(r.shape[1]):
            dma_engines[c % len(dma_engines)].dma_start(
                out=dst[:, c, :], in_=r[:, c, :])

    with ExitStack() as ctx:
        persist = ctx.enter_context(tc.tile_pool(name="persist", bufs=1))
        khT = [persist.tile([P, T], F32R, name=f"khT{i}", tag=f"khT{i}")
               for i in range(NO_T)]
        # packed V tiles: head h occupies columns [h*(HD+1), h*(HD+1)+HD),
        # with a ones column at h*(HD+1)+HD (the softmax-denominator row of
        # the PV matmul)
        vhp = [persist.tile([P, H * (HD + 1)], F32R, name=f"vh{j}",
                            tag=f"vh{j}") for j in range(NTK)]

        def vha(h, j):
            return vhp[j][:, h * (HD + 1):(h + 1) * (HD + 1)]

        ones = persist.tile([P, 1], F32, name="ones", tag="ones")
        nc.vector.memset(ones, 1.0)
        for j in range(NTK):
            v_view = vhp[j].rearrange("p (h e) -> p h e", e=HD + 1)
            nc.vector.tensor_copy(
                out=v_view[:, :, HD:HD + 1],
                in_=ones.to_broadcast((P, H, 1)))

        # weights for the interleaved chunk loop (loaded after wk/xk0 so
        # they don't delay the first k-projection matmuls)
        wpool = ctx.enter_context(tc.tile_pool(name="wq_wo", bufs=1))

        # ---- up-front phase: k and v projections ----
        with ExitStack() as actx:
            kvp = actx.enter_context(tc.tile_pool(name="kvp", bufs=1))
            kv_ps = actx.enter_context(
                tc.tile_pool(name="kv_ps", bufs=2, space="PSUM"))

            wk = kvp.tile([P, NC_T, CHL], F32R, name="wk", tag="w", bufs=2)
            load_split(wk, wkT)
            wq = wv = None

            def load_x(src, ch):
                t = kvp.tile([P, NC_T, TQ], F32R, name="xkv", tag="x", bufs=3)
                load_split(t, src[:, ch * TQ:(ch + 1) * TQ])
                return t

            # prefetch two k chunks deep; stagger weight loads in between
            xk_t = {0: load_x(kT, 0)}
            if NQ > 1:
                xk_t[1] = load_x(kT, 1)
            wv = kvp.tile([P, NC_T, CHL], F32R, name="wv", tag="w", bufs=2)
            load_split(wv, wvT)

            for ch in range(NQ):
                if ch + 2 < NQ:
                    xk_t[ch + 2] = load_x(kT, ch + 2)
                if ch == min(1, NQ - 1):
                    wq = wpool.tile([P, NC_T, CHL], F32R, name="wq", tag="wq")
                    load_split(wq, wqT)
                xk = xk_t.pop(ch)
                for o in range(NO_T):
                    ps = kv_ps.tile([P, TQ], F32, name="kps", tag="ps")
                    for c in range(NC_T):
                        nc.tensor.matmul(
                            ps, lhsT=wk[:, c, o * P:(o + 1) * P],
                            rhs=xk[:, c, :],
                            start=(c == 0), stop=(c == NC_T - 1))
                    nc.vector.tensor_copy(
                        out=khT[o][:, ch * TQ:(ch + 1) * TQ], in_=ps)

            xv_t = {0: load_x(vT, 0)}
            if NQ > 1:
                xv_t[1] = load_x(vT, 1)
            for ch in range(NQ):
                if ch + 2 < NQ:
                    xv_t[ch + 2] = load_x(vT, ch + 2)
                xv = xv_t.pop(ch)
                for jj in range(TKC):
                    j = ch * TKC + jj
                    ps = kv_ps.tile([P, CHL], F32, name="vps", tag="ps")
                    for c in range(NC_T):
                        nc.tensor.matmul(
                            ps, lhsT=xv[:, c, jj * P:(jj + 1) * P],
                            rhs=wv[:, c, :],
                            start=(c == 0), stop=(c == NC_T - 1))
                    nc.vector.tensor_copy(
                        out=vhp[j].rearrange("p (h e) -> p h e",
                                             e=HD + 1)[:, :, 0:HD],
                        in_=ps.rearrange("p (h e) -> p h e", e=HD))

        # ---- chunk loop: attention with q-proj / out-proj filler ----
        # The attention steady state is ACT(exp)-bound; the q and output
        # projection matmul groups are injected between attention groups
        # as PE filler so they run inside ACT's shadow.
        with ExitStack() as bctx:
            lpool = bctx.enter_context(tc.tile_pool(name="loop", bufs=1))
            # PSUM budget (8 banks): stA 3 + stB 2 + pv0 1 + pv1 1 + ps 1
            stA_ps = bctx.enter_context(
                tc.tile_pool(name="stA_ps", bufs=1, space="PSUM"))
            stB_ps = bctx.enter_context(
                tc.tile_pool(name="stB_ps", bufs=1, space="PSUM"))
            pv_ps = bctx.enter_context(
                tc.tile_pool(name="pv_ps", bufs=1, space="PSUM"))
            pr_ps = bctx.enter_context(
                tc.tile_pool(name="pr_ps", bufs=1, space="PSUM"))

            xq_tiles, qhc_map, mgc_map = {}, {}, {}

            def load_xq(ch):
                t = lpool.tile([P, NC_T, TQ], F32R, name="xq", tag="xq",
                               bufs=2)
                load_split(t, qT[:, ch * TQ:(ch + 1) * TQ])
                xq_tiles[ch] = t

            def qproj(hp, ch):
                ps = pr_ps.tile([P, TQ], F32, name="qps", tag="ps")
                xq = xq_tiles[ch]
                for c in range(NC_T):
                    nc.tensor.matmul(
                        ps, lhsT=wq[:, c, hp * P:(hp + 1) * P],
                        rhs=xq[:, c, :],
                        start=(c == 0), stop=(c == NC_T - 1))
                qhc = lpool.tile([P, TQ], F32R, name=f"qh{hp}",
                                 tag=f"qh{hp}", bufs=2)
                nc.vector.tensor_copy(out=qhc, in_=ps)
                qhc_map[(hp, ch)] = qhc

            def oproj_group(o, ch, pool_=None, tag="ps"):
                ps = (pool_ or pr_ps).tile([P, TQ], F32, name="ops", tag=tag)
                for c in range(NO_T):
                    nc.tensor.matmul(
                        ps, lhsT=wo[:, c, o * P:(o + 1) * P],
                        rhs=mgc_map[ch][c],
                        start=(c == 0), stop=(c == NO_T - 1))
                stg = lpool.tile([P, TQ], F32, name="stg", tag="stg", bufs=3)
                nc.vector.tensor_copy(out=stg, in_=ps)
                nc.sync.dma_start(
                    out=outT[o * P:(o + 1) * P, ch * TQ:(ch + 1) * TQ],
                    in_=stg)

            # prologue: q projections for chunk 0
            load_xq(0)
            for hp in range(H // HPT):
                qproj(hp, 0)

            # output-projection weight: first needed by chunk 1's fillers,
            # so its DMA queues behind xq0
            wo = lpool.tile([P, NO_T, C], F32R, name="wo", tag="wo")
            load_split(wo, woT)

            # flat software pipeline over every (chunk, head-pair,
            # score-group): the St matmuls of group g+1 are emitted before
            # exp/PV of group g, continuously across head-pair and chunk
            # boundaries, so the in-order PE always has independent work
            # while ACT runs exp
            pending = None      # (st_tile, group, pv_dict, is_last, hp, ch)
            use_a = True
            gi = 0
            fillers = []

            def flush_pending():
                nonlocal pending, gi
                if pending is None:
                    return
                st, group, pv, fin, hp, ch = pending
                pending = None
                n = len(group)
                e = lpool.tile([P, 3 * TQ], F32R, name="e", tag="e", bufs=2)
                nc.scalar.activation(e[:, 0:n * TQ], st[:, 0:n * TQ],
                                     ExpF, scale=scale)
                for s, (h, j) in enumerate(group):
                    nc.tensor.matmul(
                        pv[h],
                        lhsT=vha(h, j),
                        rhs=e[:, s * TQ:(s + 1) * TQ],
                        start=(j == 0), stop=(j == NTK - 1))
                gi += 1
                # inject PE filler work, but not next to a head-pair
                # boundary where it would delay the next St group and
                # starve ACT
                if fillers and gi % 2 == 0 and not fin:
                    fn, args = fillers.pop(0)
                    fn(*args)
                if fin:
                    # softmax normalization + merge for this head pair
                    for h in sorted(pv):
                        d0 = (h % HPT) * HD
                        pvc = lpool.tile([HD + 1, TQ], F32, name="pvc",
                                         tag="pvc", bufs=2)
                        nc.vector.tensor_copy(out=pvc, in_=pv[h])
                        rc = lpool.tile([1, TQ], F32, name="rc", tag="rc",
                                        bufs=2)
                        nc.vector.reciprocal(out=rc, in_=pvc[HD:HD + 1, :])
                        rb = lpool.tile([HD, TQ], F32, name="rb", tag="rb",
                                        bufs=2)
                        nc.gpsimd.partition_broadcast(rb, rc)
                        nc.vector.tensor_mul(
                            mgc_map[ch][hp][d0:d0 + HD, :], pvc[0:HD, :], rb)

            for ch in range(NQ):
                if ch + 1 < NQ:
                    load_xq(ch + 1)
                mgc_map[ch] = [
                    lpool.tile([P, TQ], F32R, name=f"mg{i}", tag=f"mg{i}",
                               bufs=2) for i in range(NO_T)]

                new_fill = []
                oo, qq = 0, 0
                while oo < (NFO if ch > 0 else 0) or \
                        qq < (H // HPT if ch + 1 < NQ else 0):
                    if oo < (NFO if ch > 0 else 0):
                        new_fill.append((oproj_group, (oo, ch - 1)))
                        oo += 1
                    if qq < (H // HPT if ch + 1 < NQ else 0):
                        new_fill.append((qproj, (qq, ch + 1)))
                        qq += 1
                fillers.extend(new_fill)

                for hp in range(H // HPT):
                    heads = [hp * HPT + i for i in range(HPT)]
                    qhc = qhc_map.pop((hp, ch))
                    slots = [(h, j) for j in range(NTK) for h in heads]
                    pv = {h: pv_ps.tile([HD + 1, TQ], F32,
                                        name=f"pv{h % HPT}",
                                        tag=f"pv{h % HPT}")
                          for h in heads}
                    g0 = 0
                    while g0 < len(slots):
                        gs = min(3 if use_a else 2, len(slots) - g0)
                        group = slots[g0:g0 + gs]
                        pool_ = stA_ps if use_a else stB_ps
                        st = pool_.tile([P, (3 if use_a else 2) * TQ], F32,
                                        name="st", tag="st")
                        for s, (h, j) in enumerate(group):
                            d0 = (h % HPT) * HD
                            nc.tensor.matmul(
                                st[:, s * TQ:(s + 1) * TQ],
                                lhsT=khT[hp][d0:d0 + HD, j * P:(j + 1) * P],
                                rhs=qhc[d0:d0 + HD, :],
                                start=True, stop=True)
                        g0 += gs
                        use_a = not use_a
                        fin = g0 >= len(slots)
                        flush_pending()
                        pending = (st, group, pv, fin, hp, ch)

            flush_pending()

            # epilogue: output projection of the last chunk, rotating psum
            # banks (attention is done, pv banks are free)
            for o in range(NFO):
                pool_, tag = [(pr_ps, "ps"), (pv_ps, "pv0"),
                              (pv_ps, "pv1")][o % 3]
                oproj_group(o, NQ - 1, pool_=pool_, tag=tag)


def build_program(*, C=D_MODEL, T=T_FULL, CHL=CH, HD=HEAD_DIM,
                  TQ=512, n_cores=N_CORES):
    nc = bacc.Bacc("TRN2", target_bir_lowering=False, debug=False,
                   enable_asserts=False, num_devices=n_cores)

    def dram(name, shape, kind, dt=F32R):
        return nc.dram_tensor(name, shape, dt, kind=kind).ap()

    qT = dram("qT", (C, T), "ExternalInput")
    kT = dram("kT", (C, T), "ExternalInput")
    vT = dram("vT", (C, T), "ExternalInput")
    wqT = dram("wqT", (C, CHL), "ExternalInput")
    wkT = dram("wkT", (C, CHL), "ExternalInput")
    wvT = dram("wvT", (C, CHL), "ExternalInput")
    woT = dram("woT", (CHL, C), "ExternalInput")
    outT = dram("outT", (C, T), "ExternalOutput", dt=F32)

    with tile.TileContext(nc) as tc:
        emit_mha(tc, outT, qT, kT, vT, wqT, wkT, wvT, woT,
                 C=C, T=T, CHL=CHL, HD=HD, TQ=TQ)
    nc.compile()
    return nc


def make_in_maps(q, k, v, Wq, Wk, Wv, Wo):
    in_maps = []
    for core in range(N_CORES):
        b, half = divmod(core, 2)
        sl = slice(half * CH, (half + 1) * CH)
        in_maps.append({
            "qT": np.ascontiguousarray(q[b].T),
            "kT": np.ascontiguousarray(k[b].T),
            "vT": np.ascontiguousarray(v[b].T),
            "wqT": np.ascontiguousarray(Wq[sl].T),
            "wkT": np.ascontiguousarray(Wk[sl].T),
            "wvT": np.ascontiguousarray(Wv[sl].T),
            "woT": np.ascontiguousarray(Wo[:, sl].T),
        })
    return in_maps


def assemble_output(results, bo):
    out = np.empty((B, T_FULL, D_MODEL), np.float32)
    bo = np.asarray(bo, np.float32)
    for b in range(B):
        acc = results[2 * b]["outT"] + results[2 * b + 1]["outT"]
        out[b] = acc.T + bo
    return out


_CACHE = {}


def run(q, k, v, Wq, Wk, Wv, Wo, bo, **spmd_kwargs):
    if "nc" not in _CACHE:
        _CACHE["nc"] = build_program()
    nc = _CACHE["nc"]
    in_maps = make_in_maps(q, k, v, Wq, Wk, Wv, Wo)
    res = bass_utils.run_bass_kernel_spmd(
        nc, in_maps, core_ids=list(range(N_CORES)), **spmd_kwargs)
    return assemble_output(res.results, bo), res


def kernel(q, k, v, Wq, Wk, Wv, Wo, bo):
    args = [np.asarray(a, np.float32)
            for a in (q, k, v, Wq, Wk, Wv, Wo, bo)]
    out, _ = run(*args)
    return out


# revision 21
# speedup vs baseline: 1.0477x; 1.0477x over previous
"""Multi-head attention (B=4, T=2048, C=1024, H=16) on 8 trn2 NeuronCores.

Sharding: core c -> (batch b = c//2, head-half = c%2, 8 heads each).
Each core computes its 8 heads' QKV projections, full attention over
T=2048, and a *partial* output projection (contraction over its 512
merged channels).  The host sums the two partials per batch and adds
the output bias (the "all-reduce after the output projection" done at
unshard time, host-side).

Device layout notes:
  - all matmuls run as float32r (full-rate fp32 on the PE array);
    every tensor feeding a matmul is declared float32r so producers
    round on write (walrus requires it)
  - scores are computed transposed St[tk, tq] so softmax needs no
    partition-axis reduction: exp runs elementwise on ACT, the
    denominator comes from a ones-column appended to the V tiles
    (PV matmul yields numerator rows 0..63 and denominator row 64)
  - no max-subtraction in softmax: scores ~ N(0,1), exp stays in fp32
    range and matches the reference numerically
  - k/v projections run up front; the q projection, attention, and
    output projection are interleaved per 512-wide query chunk so the
    ACT engine (the softmax-exp bottleneck) starts early and PE work
    fills its shadow
"""

import math
import ml_dtypes
import numpy as np
from contextlib import ExitStack

import concourse.bass as bass
import concourse.tile as tile
from concourse import bacc, mybir
from concourse import bass_utils

P = 128
F32 = mybir.dt.float32
F32R = mybir.dt.float32r
BF16 = mybir.dt.bfloat16

D_MODEL = 1024
N_HEAD = 16
HEAD_DIM = 64
B = 4
T_FULL = 2048
CH = D_MODEL // 2          # per-core merged-channel block (8 heads * 64)
N_CORES = 8


def emit_mha(tc, outT, qT, kT, vT, wqT, wkT, wvT, woT, *,
             C, T, CHL, HD, TQ=512):
    nc = tc.nc
    NC_T = C // P            # contraction tiles for qkv projections
    NO_T = CHL // P          # o-tiles of the local head block
    NTK = T // P             # key tiles
    NQ = T // TQ             # query chunks
    H = CHL // HD            # local heads
    HPT = P // HD            # heads per 128-row tile (2)
    NFO = C // P             # full-C o-tiles for the output projection
    TKC = TQ // P            # tk tiles per input chunk
    ExpF = mybir.ActivationFunctionType.Exp
    scale = 1.0 / math.sqrt(HD)

    dma_engines = [nc.sync, nc.gpsimd, nc.scalar]

    def load_split(dst, src_ap):
        """DMA a (C_sub*P, F) DRAM block into dst [P, C_sub, F], one DMA
        per c-tile spread over queues so compute can start after the
        first slice lands and transfers run in parallel."""
        r = src_ap.rearrange("(c p) f -> p c f", p=P)
        for c in range(r.shape[1]):
            dma_engines[c % len(dma_engines)].dma_start(
                out=dst[:, c, :], in_=r[:, c, :])

    with ExitStack() as ctx:
        persist = ctx.enter_context(tc.tile_pool(name="persist", bufs=1))
        khT = [persist.tile([P, T], F32R, name=f"khT{i}", tag=f"khT{i}")
               for i in range(NO_T)]
        # packed V tiles: head h occupies columns [h*(HD+1), h*(HD+1)+HD),
        # with a ones column at h*(HD+1)+HD (the softmax-denominator row of
        # the PV matmul)
        vhp = [persist.tile([P, H * (HD + 1)], F32R, name=f"vh{j}",
                            tag=f"vh{j}") for j in range(NTK)]

        def vha(h, j):
            return vhp[j][:, h * (HD + 1):(h + 1) * (HD + 1)]

        ones = persist.tile([P, 1], F32, name="ones", tag="ones")
        nc.vector.memset(ones, 1.0)
        for j in range(NTK):
            v_view = vhp[j].rearrange("p (h e) -> p h e", e=HD + 1)
            nc.vector.tensor_copy(
                out=v_view[:, :, HD:HD + 1],
                in_=ones.to_broadcast((P, H, 1)))

        # weights for the interleaved chunk loop (loaded after wk/xk0 so
        # they don't delay the first k-projection matmuls)
        wpool = ctx.enter_context(tc.tile_pool(name="wq_wo", bufs=1))

        # ---- up-front phase: k and v projections ----
        with ExitStack() as actx:
            kvp = actx.enter_context(tc.tile_pool(name="kvp", bufs=1))
            kv_ps = actx.enter_context(
                tc.tile_pool(name="kv_ps", bufs=2, space="PSUM"))

            wk = kvp.tile([P, NC_T, CHL], BF16, name="wk", tag="w", bufs=2)
            load_split(wk, wkT)
            wq = wv = None

            def load_x(src, ch):
                t = kvp.tile([P, NC_T, TQ], BF16, name="xkv", tag="x", bufs=3)
                load_split(t, src[:, ch * TQ:(ch + 1) * TQ])
                return t

            # prefetch two k chunks deep; stagger weight loads in between
            xk_t = {0: load_x(kT, 0)}
            if NQ > 1:
                xk_t[1] = load_x(kT, 1)
            wv = kvp.tile([P, NC_T, CHL], BF16, name="wv", tag="w", bufs=2)
            load_split(wv, wvT)

            for ch in range(NQ):
                if ch + 2 < NQ:
                    xk_t[ch + 2] = load_x(kT, ch + 2)
                if ch == min(1, NQ - 1):
                    wq = wpool.tile([P, NC_T, CHL], BF16, name="wq", tag="wq")
                    load_split(wq, wqT)
                xk = xk_t.pop(ch)
                for o in range(NO_T):
                    ps = kv_ps.tile([P, TQ], F32, name="kps", tag="ps")
                    for c in range(NC_T):
                        nc.tensor.matmul(
                            ps, lhsT=wk[:, c, o * P:(o + 1) * P],
                            rhs=xk[:, c, :],
                            start=(c == 0), stop=(c == NC_T - 1))
                    nc.vector.tensor_copy(
                        out=khT[o][:, ch * TQ:(ch + 1) * TQ], in_=ps)

            xv_t = {0: load_x(vT, 0)}
            if NQ > 1:
                xv_t[1] = load_x(vT, 1)
            for ch in range(NQ):
                if ch + 2 < NQ:
                    xv_t[ch + 2] = load_x(vT, ch + 2)
                xv = xv_t.pop(ch)
                for jj in range(TKC):
                    j = ch * TKC + jj
                    ps = kv_ps.tile([P, CHL], F32, name="vps", tag="ps")
                    for c in range(NC_T):
                        nc.tensor.matmul(
                            ps, lhsT=xv[:, c, jj * P:(jj + 1) * P],
                            rhs=wv[:, c, :],
                            start=(c == 0), stop=(c == NC_T - 1))
                    nc.vector.tensor_copy(
                        out=vhp[j].rearrange("p (h e) -> p h e",
                                             e=HD + 1)[:, :, 0:HD],
                        in_=ps.rearrange("p (h e) -> p h e", e=HD))

        # ---- chunk loop: attention with q-proj / out-proj filler ----
        # The attention steady state is ACT(exp)-bound; the q and output
        # projection matmul groups are injected between attention groups
        # as PE filler so they run inside ACT's shadow.
        with ExitStack() as bctx:
            lpool = bctx.enter_context(tc.tile_pool(name="loop", bufs=1))
            # PSUM budget (8 banks): stA 3 + stB 2 + pv0 1 + pv1 1 + ps 1
            stA_ps = bctx.enter_context(
                tc.tile_pool(name="stA_ps", bufs=1, space="PSUM"))
            stB_ps = bctx.enter_context(
                tc.tile_pool(name="stB_ps", bufs=1, space="PSUM"))
            pv_ps = bctx.enter_context(
                tc.tile_pool(name="pv_ps", bufs=1, space="PSUM"))
            pr_ps = bctx.enter_context(
                tc.tile_pool(name="pr_ps", bufs=1, space="PSUM"))

            xq_tiles, qhc_map, mgc_map = {}, {}, {}

            def load_xq(ch):
                t = lpool.tile([P, NC_T, TQ], BF16, name="xq", tag="xq",
                               bufs=2)
                load_split(t, qT[:, ch * TQ:(ch + 1) * TQ])
                xq_tiles[ch] = t

            def qproj(hp, ch):
                ps = pr_ps.tile([P, TQ], F32, name="qps", tag="ps")
                xq = xq_tiles[ch]
                for c in range(NC_T):
                    nc.tensor.matmul(
                        ps, lhsT=wq[:, c, hp * P:(hp + 1) * P],
                        rhs=xq[:, c, :],
                        start=(c == 0), stop=(c == NC_T - 1))
                qhc = lpool.tile([P, TQ], F32R, name=f"qh{hp}",
                                 tag=f"qh{hp}", bufs=2)
                nc.vector.tensor_copy(out=qhc, in_=ps)
                qhc_map[(hp, ch)] = qhc

            def oproj_group(o, ch, pool_=None, tag="ps"):
                ps = (pool_ or pr_ps).tile([P, TQ], F32, name="ops", tag=tag)
                for c in range(NO_T):
                    nc.tensor.matmul(
                        ps, lhsT=wo[:, c, o * P:(o + 1) * P],
                        rhs=mgc_map[ch][c],
                        start=(c == 0), stop=(c == NO_T - 1))
                stg = lpool.tile([P, TQ], F32, name="stg", tag="stg", bufs=3)
                nc.vector.tensor_copy(out=stg, in_=ps)
                nc.sync.dma_start(
                    out=outT[o * P:(o + 1) * P, ch * TQ:(ch + 1) * TQ],
                    in_=stg)

            # prologue: q projections for chunk 0
            load_xq(0)
            for hp in range(H // HPT):
                qproj(hp, 0)

            # output-projection weight: first needed by chunk 1's fillers,
            # so its DMA queues behind xq0
            wo = lpool.tile([P, NO_T, C], F32R, name="wo", tag="wo")
            load_split(wo, woT)

            # flat software pipeline over every (chunk, head-pair,
            # score-group): the St matmuls of group g+1 are emitted before
            # exp/PV of group g, continuously across head-pair and chunk
            # boundaries, so the in-order PE always has independent work
            # while ACT runs exp
            pending = None      # (st_tile, group, pv_dict, is_last, hp, ch)
            use_a = True
            gi = 0
            fillers = []

            def flush_pending():
                nonlocal pending, gi
                if pending is None:
                    return
                st, group, pv, fin, hp, ch = pending
                pending = None
                n = len(group)
                e = lpool.tile([P, 3 * TQ], F32R, name="e", tag="e", bufs=2)
                nc.scalar.activation(e[:, 0:n * TQ], st[:, 0:n * TQ],
                                     ExpF, scale=scale)
                for s, (h, j) in enumerate(group):
                    nc.tensor.matmul(
                        pv[h],
                        lhsT=vha(h, j),
                        rhs=e[:, s * TQ:(s + 1) * TQ],
                        start=(j == 0), stop=(j == NTK - 1))
                gi += 1
                # inject PE filler work, but not next to a head-pair
                # boundary where it would delay the next St group and
                # starve ACT
                if fillers and gi % 2 == 0 and not fin:
                    fn, args = fillers.pop(0)
                    fn(*args)
                if fin:
                    # softmax normalization + merge for this head pair
                    for h in sorted(pv):
                        d0 = (h % HPT) * HD
                        pvc = lpool.tile([HD + 1, TQ], F32, name="pvc",
                                         tag="pvc", bufs=2)
                        nc.vector.tensor_copy(out=pvc, in_=pv[h])
                        rc = lpool.tile([1, TQ], F32, name="rc", tag="rc",
                                        bufs=2)
                        nc.vector.reciprocal(out=rc, in_=pvc[HD:HD + 1, :])
                        rb = lpool.tile([HD, TQ], F32, name="rb", tag="rb",
                                        bufs=2)
                        nc.gpsimd.partition_broadcast(rb, rc)
                        nc.vector.tensor_mul(
                            mgc_map[ch][hp][d0:d0 + HD, :], pvc[0:HD, :], rb)

            for ch in range(NQ):
                if ch + 1 < NQ:
                    load_xq(ch + 1)
                mgc_map[ch] = [
                    lpool.tile([P, TQ], F32R, name=f"mg{i}", tag=f"mg{i}",
                               bufs=2) for i in range(NO_T)]

                new_fill = []
                oo, qq = 0, 0
                while oo < (NFO if ch > 0 else 0) or \
                        qq < (H // HPT if ch + 1 < NQ else 0):
                    if oo < (NFO if ch > 0 else 0):
                        new_fill.append((oproj_group, (oo, ch - 1)))
                        oo += 1
                    if qq < (H // HPT if ch + 1 < NQ else 0):
                        new_fill.append((qproj, (qq, ch + 1)))
                        qq += 1
                fillers.extend(new_fill)

                for hp in range(H // HPT):
                    heads = [hp * HPT + i for i in range(HPT)]
                    qhc = qhc_map.pop((hp, ch))
                    slots = [(h, j) for j in range(NTK) for h in heads]
                    pv = {h: pv_ps.tile([HD + 1, TQ], F32,
                                        name=f"pv{h % HPT}",
                                        tag=f"pv{h % HPT}")
                          for h in heads}
                    g0 = 0
                    while g0 < len(slots):
                        gs = min(3 if use_a else 2, len(slots) - g0)
                        group = slots[g0:g0 + gs]
                        pool_ = stA_ps if use_a else stB_ps
                        st = pool_.tile([P, (3 if use_a else 2) * TQ], F32,
                                        name="st", tag="st")
                        for s, (h, j) in enumerate(group):
                            d0 = (h % HPT) * HD
                            nc.tensor.matmul(
                                st[:, s * TQ:(s + 1) * TQ],
                                lhsT=khT[hp][d0:d0 + HD, j * P:(j + 1) * P],
                                rhs=qhc[d0:d0 + HD, :],
                                start=True, stop=True)
                        g0 += gs
                        use_a = not use_a
                        fin = g0 >= len(slots)
                        flush_pending()
                        pending = (st, group, pv, fin, hp, ch)

            flush_pending()

            # epilogue: output projection of the last chunk, rotating psum
            # banks (attention is done, pv banks are free)
            for o in range(NFO):
                pool_, tag = [(pr_ps, "ps"), (pv_ps, "pv0"),
                              (pv_ps, "pv1")][o % 3]
                oproj_group(o, NQ - 1, pool_=pool_, tag=tag)


def build_program(*, C=D_MODEL, T=T_FULL, CHL=CH, HD=HEAD_DIM,
                  TQ=512, n_cores=N_CORES):
    nc = bacc.Bacc("TRN2", target_bir_lowering=False, debug=False,
                   enable_asserts=False, num_devices=n_cores)

    def dram(name, shape, kind, dt=F32R):
        return nc.dram_tensor(name, shape, dt, kind=kind).ap()

    qT = dram("qT", (C, T), "ExternalInput", dt=BF16)
    kT = dram("kT", (C, T), "ExternalInput", dt=BF16)
    vT = dram("vT", (C, T), "ExternalInput", dt=BF16)
    wqT = dram("wqT", (C, CHL), "ExternalInput", dt=BF16)
    wkT = dram("wkT", (C, CHL), "ExternalInput", dt=BF16)
    wvT = dram("wvT", (C, CHL), "ExternalInput", dt=BF16)
    woT = dram("woT", (CHL, C), "ExternalInput")
    outT = dram("outT", (C, T), "ExternalOutput", dt=F32)

    with tile.TileContext(nc) as tc:
        emit_mha(tc, outT, qT, kT, vT, wqT, wkT, wvT, woT,
                 C=C, T=T, CHL=CHL, HD=HD, TQ=TQ)
    nc.compile()
    return nc


def make_in_maps(q, k, v, Wq, Wk, Wv, Wo):
    in_maps = []
    for core in range(N_CORES):
        b, half = divmod(core, 2)
        sl = slice(half * CH, (half + 1) * CH)
        bf = ml_dtypes.bfloat16
        in_maps.append({
            "qT": np.ascontiguousarray(q[b].T.astype(bf)),
            "kT": np.ascontiguousarray(k[b].T.astype(bf)),
            "vT": np.ascontiguousarray(v[b].T.astype(bf)),
            "wqT": np.ascontiguousarray(Wq[sl].T.astype(bf)),
            "wkT": np.ascontiguousarray(Wk[sl].T.astype(bf)),
            "wvT": np.ascontiguousarray(Wv[sl].T.astype(bf)),
            "woT": np.ascontiguousarray(Wo[:, sl].T),
        })
    return in_maps


def assemble_output(results, bo):
    out = np.empty((B, T_FULL, D_MODEL), np.float32)
    bo = np.asarray(bo, np.float32)
    for b in range(B):
        acc = results[2 * b]["outT"] + results[2 * b + 1]["outT"]
        out[b] = acc.T + bo
    return out


_CACHE = {}


def run(q, k, v, Wq, Wk, Wv, Wo, bo, **spmd_kwargs):
    if "nc" not in _CACHE:
        _CACHE["nc"] = build_program()
    nc = _CACHE["nc"]
    in_maps = make_in_maps(q, k, v, Wq, Wk, Wv, Wo)
    res = bass_utils.run_bass_kernel_spmd(
        nc, in_maps, core_ids=list(range(N_CORES)), **spmd_kwargs)
    return assemble_output(res.results, bo), res


def kernel(q, k, v, Wq, Wk, Wv, Wo, bo):
    args = [np.asarray(a, np.float32)
            for a in (q, k, v, Wq, Wk, Wv, Wo, bo)]
    out, _ = run(*args)
    return out


# revision 24
# speedup vs baseline: 1.0520x; 1.0042x over previous
"""Multi-head attention (B=4, T=2048, C=1024, H=16) on 8 trn2 NeuronCores.

Sharding: core c -> (batch b = c//2, head-half = c%2, 8 heads each).
Each core computes its 8 heads' QKV projections, full attention over
T=2048, and a *partial* output projection (contraction over its 512
merged channels).  The host sums the two partials per batch and adds
the output bias (the "all-reduce after the output projection" done at
unshard time, host-side).

Device layout notes:
  - projection inputs (q/k/v, Wq/Wk/Wv) are host-cast to bf16, halving
    the input DMA bytes; all other matmul tensors are float32r
    (full-rate fp32 on the PE array), declared f32r end-to-end so
    producers round on write (walrus requires it)
  - scores are computed transposed St[tk, tq] so softmax needs no
    partition-axis reduction: exp runs elementwise on ACT, the
    denominator comes from a ones-column appended to the V tiles
    (PV matmul yields numerator rows 0..63 and denominator row 64)
  - no max-subtraction in softmax: scores ~ N(0,1), exp stays in fp32
    range and matches the reference numerically
  - k/v projections run up front; the q projection, attention, and
    output projection are interleaved per 512-wide query chunk so the
    ACT engine (the softmax-exp bottleneck) starts early and PE work
    fills its shadow
"""

import math
import ml_dtypes
import numpy as np
from contextlib import ExitStack

import concourse.bass as bass
import concourse.tile as tile
from concourse import bacc, mybir
from concourse import bass_utils

P = 128
F32 = mybir.dt.float32
F32R = mybir.dt.float32r
BF16 = mybir.dt.bfloat16

D_MODEL = 1024
N_HEAD = 16
HEAD_DIM = 64
B = 4
T_FULL = 2048
CH = D_MODEL // 2          # per-core merged-channel block (8 heads * 64)
N_CORES = 8


def emit_mha(tc, outT, qT, kT, vT, wqT, wkT, wvT, woT, *,
             C, T, CHL, HD, TQ=512):
    nc = tc.nc
    NC_T = C // P            # contraction tiles for qkv projections
    NO_T = CHL // P          # o-tiles of the local head block
    NTK = T // P             # key tiles
    NQ = T // TQ             # query chunks
    H = CHL // HD            # local heads
    HPT = P // HD            # heads per 128-row tile (2)
    NFO = C // P             # full-C o-tiles for the output projection
    TKC = TQ // P            # tk tiles per input chunk
    ExpF = mybir.ActivationFunctionType.Exp
    scale = 1.0 / math.sqrt(HD)

    dma_engines = [nc.sync, nc.gpsimd, nc.scalar]

    def load_split(dst, src_ap):
        """DMA a (C_sub*P, F) DRAM block into dst [P, C_sub, F], one DMA
        per c-tile spread over queues so compute can start after the
        first slice lands and transfers run in parallel."""
        r = src_ap.rearrange("(c p) f -> p c f", p=P)
        for c in range(r.shape[1]):
            dma_engines[c % len(dma_engines)].dma_start(
                out=dst[:, c, :], in_=r[:, c, :])

    with ExitStack() as ctx:
        persist = ctx.enter_context(tc.tile_pool(name="persist", bufs=1))
        khT = [persist.tile([P, T], F32R, name=f"khT{i}", tag=f"khT{i}")
               for i in range(NO_T)]
        # packed V tiles: head h occupies columns [h*(HD+1), h*(HD+1)+HD),
        # with a ones column at h*(HD+1)+HD (the softmax-denominator row of
        # the PV matmul)
        vhp = [persist.tile([P, H * (HD + 1)], F32R, name=f"vh{j}",
                            tag=f"vh{j}") for j in range(NTK)]

        def vha(h, j):
            return vhp[j][:, h * (HD + 1):(h + 1) * (HD + 1)]

        ones = persist.tile([P, 1], F32, name="ones", tag="ones")
        nc.vector.memset(ones, 1.0)
        for j in range(NTK):
            v_view = vhp[j].rearrange("p (h e) -> p h e", e=HD + 1)
            nc.vector.tensor_copy(
                out=v_view[:, :, HD:HD + 1],
                in_=ones.to_broadcast((P, H, 1)))

        # weights for the interleaved chunk loop (loaded after wk/xk0 so
        # they don't delay the first k-projection matmuls)
        wpool = ctx.enter_context(tc.tile_pool(name="wq_wo", bufs=1))

        # ---- up-front phase: k and v projections ----
        with ExitStack() as actx:
            kvp = actx.enter_context(tc.tile_pool(name="kvp", bufs=1))
            kv_ps = actx.enter_context(
                tc.tile_pool(name="kv_ps", bufs=2, space="PSUM"))

            wk = kvp.tile([P, NC_T, CHL], BF16, name="wk", tag="w", bufs=2)
            load_split(wk, wkT)
            wq = wv = None

            def load_x(src, ch):
                t = kvp.tile([P, NC_T, TQ], BF16, name="xkv", tag="x", bufs=3)
                load_split(t, src[:, ch * TQ:(ch + 1) * TQ])
                return t

            # prefetch two k chunks deep; stagger weight loads in between
            xk_t = {0: load_x(kT, 0)}
            if NQ > 1:
                xk_t[1] = load_x(kT, 1)
            wv = kvp.tile([P, NC_T, CHL], BF16, name="wv", tag="w", bufs=2)
            load_split(wv, wvT)

            for ch in range(NQ):
                if ch + 2 < NQ:
                    xk_t[ch + 2] = load_x(kT, ch + 2)
                if ch == min(1, NQ - 1):
                    wq = wpool.tile([P, NC_T, CHL], BF16, name="wq", tag="wq")
                    load_split(wq, wqT)
                xk = xk_t.pop(ch)
                for o in range(NO_T):
                    ps = kv_ps.tile([P, TQ], F32, name="kps", tag="ps")
                    for c in range(NC_T):
                        nc.tensor.matmul(
                            ps, lhsT=wk[:, c, o * P:(o + 1) * P],
                            rhs=xk[:, c, :],
                            start=(c == 0), stop=(c == NC_T - 1))
                    nc.vector.tensor_copy(
                        out=khT[o][:, ch * TQ:(ch + 1) * TQ], in_=ps)

            xv_t = {0: load_x(vT, 0)}
            if NQ > 1:
                xv_t[1] = load_x(vT, 1)
            for ch in range(NQ):
                if ch + 2 < NQ:
                    xv_t[ch + 2] = load_x(vT, ch + 2)
                xv = xv_t.pop(ch)
                for jj in range(TKC):
                    j = ch * TKC + jj
                    ps = kv_ps.tile([P, CHL], F32, name="vps", tag="ps")
                    for c in range(NC_T):
                        nc.tensor.matmul(
                            ps, lhsT=xv[:, c, jj * P:(jj + 1) * P],
                            rhs=wv[:, c, :],
                            start=(c == 0), stop=(c == NC_T - 1))
                    nc.vector.tensor_copy(
                        out=vhp[j].rearrange("p (h e) -> p h e",
                                             e=HD + 1)[:, :, 0:HD],
                        in_=ps.rearrange("p (h e) -> p h e", e=HD))

        # ---- chunk loop: attention with q-proj / out-proj filler ----
        # The attention steady state is ACT(exp)-bound; the q and output
        # projection matmul groups are injected between attention groups
        # as PE filler so they run inside ACT's shadow.
        with ExitStack() as bctx:
            lpool = bctx.enter_context(tc.tile_pool(name="loop", bufs=1))
            # PSUM budget (8 banks): stA 3 + stB 2 + pv0 1 + pv1 1 + ps 1
            stA_ps = bctx.enter_context(
                tc.tile_pool(name="stA_ps", bufs=1, space="PSUM"))
            stB_ps = bctx.enter_context(
                tc.tile_pool(name="stB_ps", bufs=1, space="PSUM"))
            pv_ps = bctx.enter_context(
                tc.tile_pool(name="pv_ps", bufs=1, space="PSUM"))
            pr_ps = bctx.enter_context(
                tc.tile_pool(name="pr_ps", bufs=1, space="PSUM"))

            xq_tiles, qhc_map, mgc_map = {}, {}, {}

            def load_xq(ch):
                t = lpool.tile([P, NC_T, TQ], BF16, name="xq", tag="xq",
                               bufs=2)
                load_split(t, qT[:, ch * TQ:(ch + 1) * TQ])
                xq_tiles[ch] = t

            def qproj(hp, ch):
                ps = pr_ps.tile([P, TQ], F32, name="qps", tag="ps")
                xq = xq_tiles[ch]
                for c in range(NC_T):
                    nc.tensor.matmul(
                        ps, lhsT=wq[:, c, hp * P:(hp + 1) * P],
                        rhs=xq[:, c, :],
                        start=(c == 0), stop=(c == NC_T - 1))
                qhc = lpool.tile([P, TQ], F32R, name=f"qh{hp}",
                                 tag=f"qh{hp}", bufs=2)
                nc.vector.tensor_copy(out=qhc, in_=ps)
                qhc_map[(hp, ch)] = qhc

            def oproj_group(o, ch, pool_=None, tag="ps"):
                ps = (pool_ or pr_ps).tile([P, TQ], F32, name="ops", tag=tag)
                for c in range(NO_T):
                    nc.tensor.matmul(
                        ps, lhsT=wo[:, c, o * P:(o + 1) * P],
                        rhs=mgc_map[ch][c],
                        start=(c == 0), stop=(c == NO_T - 1))
                stg = lpool.tile([P, TQ], F32, name="stg", tag="stg", bufs=3)
                nc.vector.tensor_copy(out=stg, in_=ps)
                nc.sync.dma_start(
                    out=outT[o * P:(o + 1) * P, ch * TQ:(ch + 1) * TQ],
                    in_=stg)

            # prologue: only head-pair 0's q projection is needed before
            # attention starts; the rest become chunk-0 fillers
            load_xq(0)
            qproj(0, 0)

            # output-projection weight: first needed by chunk 1's fillers,
            # so its DMA queues behind xq0
            wo = lpool.tile([P, NO_T, C], F32R, name="wo", tag="wo")
            load_split(wo, woT)

            # flat software pipeline over every (chunk, head-pair,
            # score-group): the St matmuls of group g+1 are emitted before
            # exp/PV of group g, continuously across head-pair and chunk
            # boundaries, so the in-order PE always has independent work
            # while ACT runs exp
            pending = None      # (st_tile, group, pv_dict, is_last, hp, ch)
            use_a = True
            gi = 0
            fillers = []

            def flush_pending():
                nonlocal pending, gi
                if pending is None:
                    return
                st, group, pv, fin, hp, ch = pending
                pending = None
                n = len(group)
                e = lpool.tile([P, 3 * TQ], F32R, name="e", tag="e", bufs=3)
                nc.scalar.activation(e[:, 0:n * TQ], st[:, 0:n * TQ],
                                     ExpF, scale=scale)
                for s, (h, j) in enumerate(group):
                    nc.tensor.matmul(
                        pv[h],
                        lhsT=vha(h, j),
                        rhs=e[:, s * TQ:(s + 1) * TQ],
                        start=(j == 0), stop=(j == NTK - 1))
                gi += 1
                # inject PE filler work, but not next to a head-pair
                # boundary where it would delay the next St group and
                # starve ACT
                if fillers and gi % 2 == 0 and not fin:
                    fn, args = fillers.pop(0)
                    fn(*args)
                if fin:
                    # softmax normalization + merge for this head pair
                    for h in sorted(pv):
                        d0 = (h % HPT) * HD
                        pvc = lpool.tile([HD + 1, TQ], F32, name="pvc",
                                         tag="pvc", bufs=2)
                        nc.vector.tensor_copy(out=pvc, in_=pv[h])
                        rc = lpool.tile([1, TQ], F32, name="rc", tag="rc",
                                        bufs=2)
                        nc.vector.reciprocal(out=rc, in_=pvc[HD:HD + 1, :])
                        rb = lpool.tile([HD, TQ], F32, name="rb", tag="rb",
                                        bufs=2)
                        nc.gpsimd.partition_broadcast(rb, rc)
                        nc.vector.tensor_mul(
                            mgc_map[ch][hp][d0:d0 + HD, :], pvc[0:HD, :], rb)

            for ch in range(NQ):
                if ch + 1 < NQ:
                    load_xq(ch + 1)
                mgc_map[ch] = [
                    lpool.tile([P, TQ], F32R, name=f"mg{i}", tag=f"mg{i}",
                               bufs=2) for i in range(NO_T)]

                new_fill = []
                if ch == 0:
                    # remaining chunk-0 q projections, served well before
                    # their head pairs' attention begins
                    new_fill += [(qproj, (hp, 0))
                                 for hp in range(1, H // HPT)]
                oo, qq = 0, 0
                while oo < (NFO if ch > 0 else 0) or \
                        qq < (H // HPT if ch + 1 < NQ else 0):
                    if oo < (NFO if ch > 0 else 0):
                        new_fill.append((oproj_group, (oo, ch - 1)))
                        oo += 1
                    if qq < (H // HPT if ch + 1 < NQ else 0):
                        new_fill.append((qproj, (qq, ch + 1)))
                        qq += 1
                fillers.extend(new_fill)

                for hp in range(H // HPT):
                    heads = [hp * HPT + i for i in range(HPT)]
                    qhc = qhc_map.pop((hp, ch))
                    slots = [(h, j) for j in range(NTK) for h in heads]
                    pv = {h: pv_ps.tile([HD + 1, TQ], F32,
                                        name=f"pv{h % HPT}",
                                        tag=f"pv{h % HPT}")
                          for h in heads}
                    g0 = 0
                    while g0 < len(slots):
                        gs = min(3 if use_a else 2, len(slots) - g0)
                        group = slots[g0:g0 + gs]
                        pool_ = stA_ps if use_a else stB_ps
                        st = pool_.tile([P, (3 if use_a else 2) * TQ], F32,
                                        name="st", tag="st")
                        for s, (h, j) in enumerate(group):
                            d0 = (h % HPT) * HD
                            nc.tensor.matmul(
                                st[:, s * TQ:(s + 1) * TQ],
                                lhsT=khT[hp][d0:d0 + HD, j * P:(j + 1) * P],
                                rhs=qhc[d0:d0 + HD, :],
                                start=True, stop=True)
                        g0 += gs
                        use_a = not use_a
                        fin = g0 >= len(slots)
                        flush_pending()
                        pending = (st, group, pv, fin, hp, ch)

            flush_pending()

            # epilogue: output projection of the last chunk, rotating psum
            # banks (attention is done, pv banks are free)
            for o in range(NFO):
                pool_, tag = [(pr_ps, "ps"), (pv_ps, "pv0"),
                              (pv_ps, "pv1")][o % 3]
                oproj_group(o, NQ - 1, pool_=pool_, tag=tag)


def build_program(*, C=D_MODEL, T=T_FULL, CHL=CH, HD=HEAD_DIM,
                  TQ=512, n_cores=N_CORES):
    nc = bacc.Bacc("TRN2", target_bir_lowering=False, debug=False,
                   enable_asserts=False, num_devices=n_cores)

    def dram(name, shape, kind, dt=F32R):
        return nc.dram_tensor(name, shape, dt, kind=kind).ap()

    qT = dram("qT", (C, T), "ExternalInput", dt=BF16)
    kT = dram("kT", (C, T), "ExternalInput", dt=BF16)
    vT = dram("vT", (C, T), "ExternalInput", dt=BF16)
    wqT = dram("wqT", (C, CHL), "ExternalInput", dt=BF16)
    wkT = dram("wkT", (C, CHL), "ExternalInput", dt=BF16)
    wvT = dram("wvT", (C, CHL), "ExternalInput", dt=BF16)
    woT = dram("woT", (CHL, C), "ExternalInput")
    outT = dram("outT", (C, T), "ExternalOutput", dt=F32)

    with tile.TileContext(nc) as tc:
        emit_mha(tc, outT, qT, kT, vT, wqT, wkT, wvT, woT,
                 C=C, T=T, CHL=CHL, HD=HD, TQ=TQ)
    nc.compile()
    return nc


def make_in_maps(q, k, v, Wq, Wk, Wv, Wo):
    in_maps = []
    for core in range(N_CORES):
        b, half = divmod(core, 2)
        sl = slice(half * CH, (half + 1) * CH)
        bf = ml_dtypes.bfloat16
        in_maps.append({
            "qT": np.ascontiguousarray(q[b].T.astype(bf)),
            "kT": np.ascontiguousarray(k[b].T.astype(bf)),
            "vT": np.ascontiguousarray(v[b].T.astype(bf)),
            "wqT": np.ascontiguousarray(Wq[sl].T.astype(bf)),
            "wkT": np.ascontiguousarray(Wk[sl].T.astype(bf)),
            "wvT": np.ascontiguousarray(Wv[sl].T.astype(bf)),
            "woT": np.ascontiguousarray(Wo[:, sl].T),
        })
    return in_maps


def assemble_output(results, bo):
    out = np.empty((B, T_FULL, D_MODEL), np.float32)
    bo = np.asarray(bo, np.float32)
    for b in range(B):
        acc = results[2 * b]["outT"] + results[2 * b + 1]["outT"]
        out[b] = acc.T + bo
    return out


_CACHE = {}


def run(q, k, v, Wq, Wk, Wv, Wo, bo, **spmd_kwargs):
    if "nc" not in _CACHE:
        _CACHE["nc"] = build_program()
    nc = _CACHE["nc"]
    in_maps = make_in_maps(q, k, v, Wq, Wk, Wv, Wo)
    res = bass_utils.run_bass_kernel_spmd(
        nc, in_maps, core_ids=list(range(N_CORES)), **spmd_kwargs)
    return assemble_output(res.results, bo), res


def kernel(q, k, v, Wq, Wk, Wv, Wo, bo):
    args = [np.asarray(a, np.float32)
            for a in (q, k, v, Wq, Wk, Wv, Wo, bo)]
    out, _ = run(*args)
    return out
